# revision 57
# baseline (speedup 1.0000x reference)
"""Trainium2 Bass kernel for bidirectional OTAM soft-DTW over CLIP frame features.

Computes, for query features X [512,16,512] and support features Y [128,16,512]:
  sims = cos_sim(frames) -> dists = 1 - sims -> cum = OTAM_DP(dists) + OTAM_DP(dists.T)
returning cum [512, 128].

Strategy (per core, 8-way data parallel over the 512 queries; 64 q x 128 s each):
  - All tensor data bf16: X/Y enter SBUF via gpsimd software-DGE cast DMAs
    (billed at the bf16 output size; half the HBM-load cost of f32 loads).
  - Row norms: fused square+accumulate (DVE scalar_tensor_tensor / Act Square,
    split across both engines); rsqrt via constant-seed Newton on DVE (the
    norms are chi^2_512-concentrated) -> no Sqrt table, Act keeps one table.
  - cos via bf16 matmuls (f32 PSUM accumulate): psum[s,(q,tq)] per support
    frame ts; W = exp(2cos-2) via Act exp with per-partition scale 2/|y|.
  - W written twice: exp -> G[s,q,tq,ts] (dir2 row l = stride-18 flat view at
    ts=l+1) and Pool gather-copy -> G1[s,sec=tq-1,(q,ts)] (dir1 row l =
    contiguous sec l).  DP rows are single tensor_tensor_scan instructions
    (state = (data0 + state) * data1) batched over (q, m); segment resets
    ride on W=0 edge columns; the two DP edge terms are ONE tensor_tensor
    fixup per row using a 2.0-constant strip baked into the E-row tiles.
  - dir2 pipelines behind the per-ts matmul+exp cascade (scans are DVE-only
    on TRN2 -- gpsimd rejects the scan opcode); dir1 is the serial DVE tail.
    All fixups stay on DVE: during the phase the rows are PE-paced with
    slack, and the tail avoids cross-engine semaphore round-trips.
  - Scheduling notes: input tiles are split per DMA chunk and normalized X
    gets its own tiles, so consumers carry exactly one cross-engine wait
    (avoiding hoisted head-of-queue relay semaphores); bulk memsets are
    emitted last so the list scheduler uses them as idle filler; rsqrt
    Newton chains for the exp scales run on Pool.
"""

import sys

for _p in ("/opt/trn_rl_repo", "/opt/pypackages"):
    if _p not in sys.path:
        sys.path.append(_p)

import numpy as np

import concourse.bass as bass
import concourse.bacc as bacc
import concourse.mybir as mybir
import concourse.tile as tile
from concourse.ap import AP
from concourse.bass_utils import run_bass_kernel_spmd

F32 = mybir.dt.float32
BF16 = mybir.dt.bfloat16
AF = mybir.ActivationFunctionType
ALU = mybir.AluOpType

S, Q, T, D = 128, 512, 16, 512
NCORES = 8
QC = Q // NCORES          # 64 queries per core
M = T + 2                 # 18: padded DP width
KC = D // 128             # 4 contraction chunks
SEG = QC * M              # 1152 flat scan length
EOFF = SEG + 1            # E-row offset inside an E tile (strip + pad first)


def _fv(t, offset, stride, count):
    """[128, count] flat view of tile t's free dim."""
    return AP(t.tensor, t.offset + offset, [list(t.ap[0]), [stride, count]])


def build_kernel() -> bass.Bass:
    nc = bacc.Bacc(None)
    tf = nc.dram_tensor("tf", [QC, T, D], F32, kind="ExternalInput")
    sf = nc.dram_tensor("sf", [S, T, D], F32, kind="ExternalInput")
    out = nc.dram_tensor("out", [S, QC], F32, kind="ExternalOutput")

    with tile.TileContext(nc) as tc:
        with (
            tc.tile_pool(name="big", bufs=1) as big,
            tc.tile_pool(name="small", bufs=1) as small,
            tc.tile_pool(name="psum", bufs=3, space="PSUM") as psum,
            tc.tile_pool(name="wpsum", bufs=1, space="PSUM") as wpsum,
        ):
            # ---- persistent tiles
            Xc = [
                big.tile([128, 2, D], BF16, name=f"xc{c}", tag=f"xc{c}")
                for c in range(4)
            ]  # X rows (q,t), 2 row-tiles per chunk
            Xn = [
                big.tile([128, 2, D], BF16, name=f"xn{c}", tag=f"xn{c}")
                for c in range(4)
            ]  # normalized X (separate tile: transposes then depend only on
            #    the DVE mul, avoiding a head-of-queue relay wait on SP)
            Yc = [
                big.tile([128, 8, D], BF16, name=f"yc{h}", tag=f"yc{h}")
                for h in range(2)
            ]  # Y rows s, 8 ts per chunk
            XbfT = big.tile([128, KC, QC * T], BF16, tag="XbfT")    # [d, (q,t)]
            YbfT = big.tile([128, KC, S * T], BF16, tag="YbfT")     # [d, ts*128+s]
            G = big.tile([128, T, QC, M], BF16, tag="G")            # [s,ts,q,tq]
            G1 = big.tile([128, T, SEG], BF16, tag="G1")            # [s,sec,(q,ts)]
            Z0 = big.tile([128, SEG], BF16, tag="Z0")
            # E tiles: [2.0-strip (SEG) | pad (1) | row (SEG)]; the strip
            # provides a per-q 2.0 constant at stride M so both DP edge
            # fixups collapse into one tensor_tensor add (see fixup()).
            Eb = [
                big.tile([128, SEG + 1 + SEG], BF16, name=f"eb{i}", tag=f"eb{i}")
                for i in range(4)
            ]
            nx = small.tile([128, 8], F32, tag="nx")     # |x|^2 per row tile
            ny = small.tile([128, T], F32, tag="ny")     # 0.25*|y|^2 per ts
            rx = small.tile([128, 8], F32, tag="rx")     # 1/|x|
            sy = small.tile([128, T], F32, tag="sy")     # 2/|y|
            qt1 = small.tile([128, T], F32, tag="qt1")
            qt2 = small.tile([128, T], F32, tag="qt2")
            bm2 = small.tile([128, 1], F32, tag="bm2")
            l1 = small.tile([128, QC], F32, tag="l1")
            l2 = small.tile([128, QC], F32, tag="l2")
            res = small.tile([128, QC], F32, tag="res")
            ysq = small.tile([128, D], BF16, tag="ysq")
            xsq_d = small.tile([128, D], BF16, tag="xsq_d")
            xsq_a = small.tile([128, D], BF16, tag="xsq_a")
            dum = small.tile([128, 1], F32, tag="dum")

            def newton_rsqrt(dst, n, w, seed, eng=None):
                """dst = 1/sqrt(n) without an activation table.  n is
                chi^2_512-concentrated around seed^-2, so a constant seed +
                2 Newton steps reaches ~0.5% rel (cos scale error ~1e-4 abs,
                far inside the 2e-2 gate)."""
                if eng is None:
                    eng = nc.vector
                eng.memset(dst, seed)
                for _ in range(2):  # r *= 1.5 - 0.5*n*r^2
                    eng.tensor_tensor(w, dst, dst, ALU.mult)
                    eng.tensor_tensor(w, w, n, ALU.mult)
                    eng.tensor_scalar_mul(w, w, -0.5)
                    eng.tensor_scalar_add(w, w, 1.5)
                    eng.tensor_tensor(dst, dst, w, ALU.mult)

            # ---- dummy act first so the table load runs during the DMAs
            # instead of gating the first real activation
            nc.vector.memset(dum[:], 1.0)
            nc.scalar.activation(dum[:], dum[:], AF.Square)
            nc.vector.memset(bm2[:], -2.0)
            # PE p-state warm-up: a chain of throwaway matmuls keeps the
            # tensor engine busy through the prep window so the ramp (0.65 ->
            # 2.4 GHz over ~3us of activity) completes before the first real
            # matmul.  Sized to end ~1.5us before the real stream starts: if
            # the p-state survives the short gap this saves the ~1.6us ramp
            # tax on ts0 (which gates the whole dir2 cascade); if not, the PE
            # was idle here anyway.
            wt = small.tile([128, D], BF16, tag="wt")
            nc.vector.memset(wt[:], 0.001)
            wps = wpsum.tile([128, D], F32, tag="wps")
            for _ in range(40):
                nc.tensor.matmul(wps[:], wt[:, 0:128], wt[:], start=True, stop=True)

            # ---- Pool: cast-load X (4 chunks of 2 row-tiles), Y (4 chunks)
            tf_r = tf.rearrange("q t d -> (q t) d").rearrange(
                "(k p) d -> p k d", p=128
            )
            def x_dma(c):
                nc.gpsimd.dma_start(out=Xc[c][:], in_=tf_r[:, 2 * c : 2 * c + 2, :])

            def y_dma(h):
                nc.gpsimd.dma_start(out=Yc[h][:], in_=sf[:, 8 * h : 8 * h + 8, :])

            for c in range(4):
                x_dma(c)
            for h in range(2):
                y_dma(h)

            # ---- X norms split DVE (k 0,1,6,7) / Act (k 2..5)
            def x_norm_dve(k):
                xt = Xc[k // 2][:, k % 2, :]
                nc.vector.scalar_tensor_tensor(
                    xsq_d[:], xt, 1.0, xt, ALU.bypass, ALU.mult,
                    accum_out=nx[:, k : k + 1],
                )

            def x_norm_act(k):
                nc.scalar.activation(
                    xsq_a[:], Xc[k // 2][:, k % 2, :], AF.Square,
                    accum_out=nx[:, k : k + 1],
                )

            x_norm_dve(0)
            x_norm_dve(1)
            for k in (2, 3, 4, 5):
                x_norm_act(k)
            x_norm_dve(6)
            x_norm_dve(7)

            def y_norm(ts):  # Act: ny[ts] = sum(Square(0.5*y)) = 0.25|y|^2
                nc.scalar.activation(
                    ysq[:], Yc[ts // 8][:, ts % 8, :], AF.Square, scale=0.5,
                    accum_out=ny[:, ts : ts + 1],
                )

            def y_tp(eng, ts0, ts1):
                for ts in range(ts0, ts1):
                    for c in range(KC):
                        eng.dma_start(
                            out=YbfT[:, c, ts * 128 : (ts + 1) * 128],
                            in_=Yc[ts // 8][:, ts % 8, :][:, c * 128 : (c + 1) * 128],
                            transpose=True,
                        )

            y_tp(nc.sync, 0, 2)  # ts0-1 feed the first matmuls

            newton_rsqrt(rx[:], nx[:], qt1[:, 0:8], 512.0 ** -0.5)
            # c-major transpose order: matmul K-chunk c unblocks after just
            # 4 X transposes instead of all 16
            for kg in (range(4), range(4, 8)):
                for k in kg:
                    xt = Xn[k // 2][:, k % 2, :]
                    nc.vector.tensor_scalar_mul(
                        xt, Xc[k // 2][:, k % 2, :], rx[:, k : k + 1]
                    )
                for c in range(KC):
                    for k in kg:
                        xt = Xn[k // 2][:, k % 2, :]
                        nc.sync.dma_start(
                            out=XbfT[:, c, k * 128 : (k + 1) * 128],
                            in_=xt[:, c * 128 : (c + 1) * 128],
                            transpose=True,
                        )
            y_tp(nc.sync, 2, 16)

            # ---- matmul + exp + G1 gather per ts; Y norms ride Act gaps
            for ts in range(4):
                y_norm(ts)
            newton_rsqrt(sy[:, 0:4], ny[:, 0:4], qt2[:, 0:4], 128.0 ** -0.5,
                         eng=nc.gpsimd)
            for ts in range(T):
                if ts < 12:  # stay one 4-group of Y scales ahead of the exps
                    y_norm(ts + 4)
                    if ts % 4 == 3:
                        g = ts + 1
                        newton_rsqrt(
                            sy[:, g : g + 4], ny[:, g : g + 4],
                            qt2[:, g : g + 4], 128.0 ** -0.5, eng=nc.gpsimd,
                        )
                ps = psum.tile([128, QC * T], F32, tag="ps", name=f"ps{ts}")
                for h in range(2):
                    for c in range(KC):
                        nc.tensor.matmul(
                            ps[:, h * 512 : (h + 1) * 512],
                            YbfT[:, c, ts * 128 : (ts + 1) * 128],
                            XbfT[:, c, h * 512 : (h + 1) * 512],
                            start=(c == 0),
                            stop=(c == KC - 1),
                        )
                psv = ps.rearrange("p (q t) -> p q t", t=T)
                nc.scalar.activation(
                    G[:, ts, :, 1 : M - 1], psv, AF.Exp,
                    bias=bm2[:], scale=sy[:, ts : ts + 1],
                )
                if ts < T - 1:
                    g1dst = AP(
                        G1.tensor, G1.offset + ts + 1,
                        [list(G1.ap[0]), [M, QC], [SEG, T]],
                    )
                    nc.gpsimd.tensor_copy(g1dst, G[:, ts, :, 1 : M - 1])
                else:
                    # split the last gather so dir1 row 0 only waits secs 0-3
                    for s0, s1 in ((0, 4), (4, T)):
                        g1dst = AP(
                            G1.tensor, G1.offset + ts + 1 + s0 * SEG,
                            [list(G1.ap[0]), [M, QC], [SEG, s1 - s0]],
                        )
                        nc.gpsimd.tensor_copy(
                            g1dst, G[:, ts, :, 1 + s0 : 1 + s1]
                        )

            # ---- bulk memsets: emitted late (low scheduler priority) so
            # they fill engine idle slots instead of delaying the X/Y chains
            nc.vector.memset(Z0[:], 0.0)
            nc.vector.memset(_fv(Z0, 1, M, QC), 1.0)   # data0[m=1] = 1 per q
            for e in Eb:
                nc.vector.memset(_fv(e, 0, M, QC), 2.0)  # fixup strip
                nc.vector.memset(e[:, SEG : SEG + 1], 0.0)  # shift pad
            nc.vector.memset(G[:, :, :, 0], 0.0)         # dir2 segment reset
            nc.gpsimd.memset(G[:, :, :, M - 1], 1.0)     # dir2 pad col
            g1v = G1.rearrange("p s (q m) -> p s q m", m=M)
            nc.gpsimd.memset(g1v[:, :, :, 0], 0.0)       # dir1 segment reset
            nc.gpsimd.memset(g1v[:, :, :, M - 1], 1.0)   # dir1 pad col

            # ---- DP rows: state=(data0+state)*data1 over flat (q, m=0..17).
            # After each row, one fused fixup prepares it as next row's data0:
            #   E[q,0]  <- E[q,1] + 2.0   (edge m=1: cur E[0]=1 + prev E[0]=1)
            #   E[q,16] <- E[q,16] + E[q,17]   (edge m=17 extra predecessor)
            part = None

            def fixup(eng, cur):
                p0 = list(cur.ap[0])
                o = cur.offset
                fo = AP(cur.tensor, o + EOFF, [p0, [M, QC], [16, 2]])
                fi0 = AP(cur.tensor, o + EOFF + 1, [p0, [M, QC], [16, 2]])
                fi1 = AP(cur.tensor, o, [p0, [M, QC], [SEG + 17, 2]])
                eng.tensor_tensor(fo, fi0, fi1, ALU.add)

            def rows(dir_idx, data1_of, fix_eng):
                e_a, e_b = Eb[2 * dir_idx], Eb[2 * dir_idx + 1]
                prev = None
                for l in range(T):
                    cur = e_a if l % 2 == 0 else e_b
                    d0 = (
                        _fv(Z0, 0, 1, SEG) if l == 0 else _fv(prev, SEG, 1, SEG)
                    )
                    nc.vector.tensor_tensor_scan(
                        _fv(cur, EOFF, 1, SEG), d0, data1_of(l),
                        0.0, ALU.add, ALU.mult,
                    )
                    if l < T - 1:
                        fixup(fix_eng, cur)
                    prev = cur
                return prev

            # dir2 pipelines with the exps; fixups on Pool (latency hidden)
            last2 = rows(1, lambda l: _fv(G, l * SEG, 1, SEG), nc.vector)
            nc.scalar.activation(l2[:], _fv(last2, EOFF + 17, M, QC), AF.Ln)
            nc.gpsimd.tensor_scalar_mul(l2[:], l2[:], -0.5)

            # dir1: the serial tail; fixups on DVE (chain-latency critical)
            last1 = rows(0, lambda l: _fv(G1, l * SEG, 1, SEG), nc.vector)

            # ---- epilogue: cum = -0.5*(ln E1[17] + ln E2[17])
            nc.scalar.activation(l1[:], _fv(last1, EOFF + 17, M, QC), AF.Ln)
            nc.vector.scalar_tensor_tensor(
                res[:], l1[:], -0.5, l2[:], ALU.mult, ALU.add
            )
            nc.sync.dma_start(out=out[:], in_=res[:])

    nc.compile()
    return nc


_NC_CACHE: list = []


def kernel(support_features: np.ndarray, target_features: np.ndarray) -> np.ndarray:
    sfv = np.ascontiguousarray(np.asarray(support_features, dtype=np.float32))
    tfv = np.ascontiguousarray(np.asarray(target_features, dtype=np.float32))
    assert sfv.shape == (S, T, D) and tfv.shape == (Q, T, D)

    if not _NC_CACHE:
        _NC_CACHE.append(build_kernel())
    nc = _NC_CACHE[0]

    in_maps = [{"tf": tfv[i * QC : (i + 1) * QC], "sf": sfv} for i in range(NCORES)]
    r = run_bass_kernel_spmd(nc, in_maps, list(range(NCORES))).results
    full = np.empty((Q, S), np.float32)
    for i in range(NCORES):
        full[i * QC : (i + 1) * QC, :] = r[i]["out"].T
    return full
